# revision 1
# baseline (speedup 1.0000x reference)
"""Trainium2 Bass kernel for nn_MatchLoss.

Reference computation:
    an, bn, cn = l1_normalize(a|b|c, dim=C)        # per (b, h, w) column
    sim_ab = einsum('bchw,bcij->bhwij', an, bn)
    sim_ac = einsum('bchw,bcij->bhwij', an, cn)
    out = mean(|sim_ac - sim_ab|)                   # scalar

Algebraic restructure used here (per batch, flattening hw -> 4096):
    sim_ac - sim_ab = an^T @ (cn - bn) = diag(1/na) @ (a^T @ D),
        D = c * diag(1/nc) - b * diag(1/nb)
    loss_part = sum_q (1/na[q]) * sum_p |(a^T D)[q, p]|
so `a` is never normalized on-device; its norm is applied to the
per-query rowsums after the abs-reduce.

Sharding: 8 cores = 2 batches x 4 slices of the p (=ij) axis.  Each core
gets the full `a` for its batch (128 x 4096) plus a 1024-column slice of
b and c, computes rowsums of |a^T D_slice| scaled by 1/na into a
(128 x 32) partial, and the host sums the 8 partials.

Matmul inputs are bf16 (PE at 1 cycle/row); all accumulation (PSUM,
rowsums, norms) is fp32.  The only bf16 roundings are of |x| before the
norm sums, of 1/nb, 1/nc before the column scaling, and of a and D
before the big matmul - each perturbs the 33.5M-term mean by ~1e-4
relative, far inside fp32-envelope tolerances.
"""

import numpy as np

try:
    import concourse.bacc as bacc
    import concourse.tile as tile
    import concourse.mybir as mybir
    from concourse import bass_utils
except ImportError:  # pragma: no cover - fallback for bare containers
    import sys

    sys.path.insert(0, "/opt/trn_rl_repo")
    import concourse.bacc as bacc
    import concourse.tile as tile
    import concourse.mybir as mybir
    from concourse import bass_utils

B, C, H, W = 2, 128, 64, 64
HW = H * W              # 4096 (q axis, and full p axis)
N_CORES = 8
PSL = HW // 4           # 1024: per-core p-slice
QT = 128                # q tile (partition dim of PSUM result)
NQT = HW // QT          # 32 q tiles
CH = 512                # matmul moving chunk (one PSUM bank of fp32)

_F32 = mybir.dt.float32
_BF16 = mybir.dt.bfloat16
_AX = mybir.AxisListType
_AF = mybir.ActivationFunctionType
_OP = mybir.AluOpType


def _emit(tc, a_d, b_d, c_d, o_d):
    nc = tc.nc

    import contextlib

    with contextlib.ExitStack() as ctx:
        ctx.enter_context(
            nc.allow_low_precision(
                reason="bf16 matmul inputs; all accumulation stays fp32"
            )
        )
        sb = ctx.enter_context(tc.tile_pool(name="sb", bufs=1))

        A = sb.tile([C, HW], _BF16)
        absA = sb.tile([C, HW], _BF16)
        Bs = sb.tile([C, PSL], _F32)
        Cs = sb.tile([C, PSL], _F32)
        absB = sb.tile([C, PSL], _BF16)
        absC = sb.tile([C, PSL], _BF16)
        D = sb.tile([C, PSL], _BF16)
        t1 = sb.tile([C, PSL], _F32)
        t2 = sb.tile([C, PSL], _F32)
        ones_col = sb.tile([C, 1], _BF16)
        ones_row = sb.tile([1, C], _BF16)
        zeros_col = sb.tile([C, 1], _F32)
        rows = sb.tile([1, 2 * PSL], _BF16)   # [1/nb | 1/nc]
        rna = sb.tile([C, NQT], _F32)
        rs_dve = sb.tile([C, NQT], _F32)
        rs_act = sb.tile([C, NQT], _F32)
        rs_sum = sb.tile([C, NQT], _F32)
        res = sb.tile([C, NQT], _F32)
        trash = sb.tile([C, PSL], _BF16)

        # --- input DMAs (b/c first: they gate the critical path to D);
        # split so abs/norm can start on the first half early ---
        for j in range(PSL // CH):
            sl = slice(j * CH, (j + 1) * CH)
            nc.sync.dma_start(Bs[:, sl], b_d[:, sl])
        for j in range(PSL // CH):
            sl = slice(j * CH, (j + 1) * CH)
            nc.sync.dma_start(Cs[:, sl], c_d[:, sl])

        nc.vector.memset(ones_col[:], 1.0)
        nc.vector.memset(ones_row[:], 1.0)
        nc.vector.memset(zeros_col[:], 0.0)
        nc.gpsimd.memset(rs_dve[:], 0.0)
        nc.gpsimd.memset(rs_act[:], 0.0)

        # --- |b|, |c| on ACT (bf16 outputs feeding the norm matmuls) ---
        for j in range(PSL // CH):
            sl = slice(j * CH, (j + 1) * CH)
            nc.scalar.activation(absB[:, sl], Bs[:, sl], _AF.Abs, bias=zeros_col[:])
        for j in range(PSL // CH):
            sl = slice(j * CH, (j + 1) * CH)
            nc.scalar.activation(absC[:, sl], Cs[:, sl], _AF.Abs, bias=zeros_col[:])

        na_ps = ctx.enter_context(tc.tile_pool(name="na_ps", bufs=1, space="PSUM"))
        na = na_ps.tile([C, NQT], _F32)

        with (
            tc.tile_pool(name="rows_ps", bufs=1, space="PSUM") as rows_ps,
            tc.tile_pool(name="bc_ps", bufs=2, space="PSUM") as bc_ps,
        ):
            # column L1 norms of b,c: ones^T @ |x| -> one (1, 2*PSL) row
            nrow = rows_ps.tile([1, 2 * PSL], _F32)
            for j in range(PSL // CH):
                sl = slice(j * CH, (j + 1) * CH)
                nc.tensor.matmul(
                    nrow[0:1, sl], lhsT=ones_col[:], rhs=absB[:, sl],
                    start=True, stop=True,
                )
            nc.vector.reciprocal(rows[0:1, 0:PSL], nrow[0:1, 0:PSL])
            for j in range(PSL // CH):
                sl = slice(j * CH, (j + 1) * CH)
                sl_c = slice(PSL + j * CH, PSL + (j + 1) * CH)
                nc.tensor.matmul(
                    nrow[0:1, sl_c], lhsT=ones_col[:], rhs=absC[:, sl],
                    start=True, stop=True,
                )
            nc.vector.reciprocal(rows[0:1, PSL:], nrow[0:1, PSL:])

            for i in range(4):
                sl_a = slice(i * PSL, (i + 1) * PSL)
                nc.gpsimd.dma_start(A[:, sl_a], a_d[:, sl_a])  # f32->bf16

            # |a| on ACT -- held past the critical absB/absC passes so the
            # in-order ACT queue can't head-of-line block on the a DMA
            with tc.tile_wait_until(0.0067):
                for i in range(4):
                    sl_a = slice(i * PSL, (i + 1) * PSL)
                    nc.scalar.activation(
                        absA[:, sl_a], A[:, sl_a], _AF.Abs, bias=zeros_col[:]
                    )

            # broadcast 1/nb, 1/nc across partitions (K=1 outer product),
            # then D = b * rb - c * rc (bf16 out)
            for j in range(PSL // CH):
                sl = slice(j * CH, (j + 1) * CH)
                sl_c = slice(PSL + j * CH, PSL + (j + 1) * CH)
                rb_bc = bc_ps.tile([C, CH], _F32, tag="bc")
                nc.tensor.matmul(
                    rb_bc[:], lhsT=ones_row[:], rhs=rows[0:1, sl],
                    start=True, stop=True,
                )
                rc_bc = bc_ps.tile([C, CH], _F32, tag="bc")
                nc.tensor.matmul(
                    rc_bc[:], lhsT=ones_row[:], rhs=rows[0:1, sl_c],
                    start=True, stop=True,
                )
                nc.vector.tensor_mul(t1[:, sl], Bs[:, sl], rb_bc[:])
                nc.vector.tensor_mul(t2[:, sl], Cs[:, sl], rc_bc[:])
                nc.vector.tensor_sub(D[:, sl], t1[:, sl], t2[:, sl])

        # --- main loop: M = a_tile^T @ D, rowsum(|M|) on DVE/ACT alternately;
        # na matmuls (tiny) ride along on the PE once |a| chunks are ready ---
        with tc.tile_pool(name="m_ps", bufs=3, space="PSUM") as m_ps:
            for t in range(NQT):
                M = m_ps.tile([C, PSL], _F32)
                for j in range(PSL // CH):
                    sl = slice(j * CH, (j + 1) * CH)
                    nc.tensor.matmul(
                        M[:, sl],
                        lhsT=A[:, t * QT : (t + 1) * QT],
                        rhs=D[:, sl],
                        start=True,
                        stop=True,
                    )
                if t % 2 == 0:
                    nc.vector.tensor_reduce(
                        out=rs_dve[:, t : t + 1],
                        in_=M[:],
                        axis=_AX.X,
                        op=_OP.add,
                        apply_absolute_value=True,
                    )
                else:
                    nc.scalar.activation(
                        trash[:],
                        M[:],
                        _AF.Abs,
                        bias=zeros_col[:],
                        accum_out=rs_act[:, t : t + 1],
                    )

        # na matmuls are tiny; schedule them into main-loop PE gaps
        with tc.tile_wait_until(0.012):
            for tn in range(NQT):
                nc.tensor.matmul(
                    na[:, tn : tn + 1],
                    lhsT=absA[:, tn * QT : (tn + 1) * QT],
                    rhs=ones_col[:],
                    start=True,
                    stop=True,
                )

        # --- tail: combine rowsums, scale by 1/na, write out ---
        nc.vector.reciprocal(rna[:], na[:])
        nc.vector.tensor_add(rs_sum[:], rs_dve[:], rs_act[:])
        nc.vector.tensor_mul(res[:], rs_sum[:], rna[:])
        nc.sync.dma_start(o_d, res[:])


def _build():
    nc = bacc.Bacc(
        "TRN2", target_bir_lowering=False, debug=False, num_devices=N_CORES
    )
    a_d = nc.dram_tensor("a_full", (C, HW), _F32, kind="ExternalInput").ap()
    b_d = nc.dram_tensor("b_sl", (C, PSL), _F32, kind="ExternalInput").ap()
    c_d = nc.dram_tensor("c_sl", (C, PSL), _F32, kind="ExternalInput").ap()
    o_d = nc.dram_tensor("out", (C, NQT), _F32, kind="ExternalOutput").ap()
    with tile.TileContext(nc) as tc:
        _emit(tc, a_d, b_d, c_d, o_d)
    nc.finalize()
    return nc


_NC_CACHE = {}


def _get_nc():
    if "nc" not in _NC_CACHE:
        _NC_CACHE["nc"] = _build()
    return _NC_CACHE["nc"]


def _in_maps(a, b, c):
    a = np.ascontiguousarray(np.asarray(a, dtype=np.float32).reshape(B, C, HW))
    b = np.ascontiguousarray(np.asarray(b, dtype=np.float32).reshape(B, C, HW))
    c = np.ascontiguousarray(np.asarray(c, dtype=np.float32).reshape(B, C, HW))
    maps = []
    for core in range(N_CORES):
        bi, pi = divmod(core, 4)
        sl = slice(pi * PSL, (pi + 1) * PSL)
        maps.append(
            {
                "a_full": a[bi],
                "b_sl": np.ascontiguousarray(b[bi, :, sl]),
                "c_sl": np.ascontiguousarray(c[bi, :, sl]),
            }
        )
    return maps


def kernel(a, b, c):
    nc = _get_nc()
    res = bass_utils.run_bass_kernel_spmd(
        nc, _in_maps(a, b, c), core_ids=list(range(N_CORES))
    )
    total = np.float64(0.0)
    for core in range(N_CORES):
        total += np.sum(res.results[core]["out"], dtype=np.float64)
    return np.float32(total / (B * HW * HW))



# revision 9
# speedup vs baseline: 1.0166x; 1.0166x over previous
"""Trainium2 Bass kernel for nn_MatchLoss.

Reference computation:
    an, bn, cn = l1_normalize(a|b|c, dim=C)        # per (b, h, w) column
    sim_ab = einsum('bchw,bcij->bhwij', an, bn)
    sim_ac = einsum('bchw,bcij->bhwij', an, cn)
    out = mean(|sim_ac - sim_ab|)                   # scalar

Restructure (per batch, hw -> 4096):
    sim_ac - sim_ab = diag(1/na) @ (a^T @ D),  D = c*diag(1/nc) - b*diag(1/nb)
    loss_part = sum_q (1/na[q]) * sum_p |(a^T D)[q, p]|

Sharding: 8 cores = 2 batches x 4 slices of the p axis.  Each core gets
full `a` for its batch plus a 1024-column slice of b and c (packed as one
dram tensor [b0|c0|b1|c1] in 512-col chunks), computes 1/na-scaled
rowsums of |a^T D| into a (128 x 32) partial; host sums the partials.

Schedule highlights (everything tuned against the TRN2 TimelineSim cost
model):
  * all loads are f32->bf16 casting DMAs on the gpsimd queue; the SWDGE
    descriptor generation runs before the Pool engine has real work
  * PE warmup matmuls at t=0 beat the p-state ramp (0.65 -> 2.4 GHz)
  * norms are computed partition-major ([128,16] reciprocal, ~0.3us,
    vs 2.1us for a [1,2048] single-lane reciprocal), then PE-transposed
    to rows for the rank-1 broadcast matmuls
  * the dominant abs-rowsum of M = a^T D is split across THREE engines:
    DVE tensor_reduce, ACT activation(Abs)+accum, Pool tensor_scalar
    (abs_max, 0) + add-accum, weighted by their measured per-tile costs
  * the first SPLIT q-tiles reduce each 512-col p-chunk separately so
    the loop can start before the second half of b/c has even landed
"""

import numpy as np

try:
    import concourse.bacc as bacc
    import concourse.tile as tile
    import concourse.mybir as mybir
    from concourse import bass_utils
except ImportError:  # pragma: no cover - fallback for bare containers
    import sys

    sys.path.insert(0, "/opt/trn_rl_repo")
    import concourse.bacc as bacc
    import concourse.tile as tile
    import concourse.mybir as mybir
    from concourse import bass_utils

B, C, H, W = 2, 128, 64, 64
HW = H * W              # 4096 (q axis, and full p axis)
N_CORES = 8
PSL = HW // 4           # 1024: per-core p-slice
QT = 128                # q tile (partition dim of PSUM result)
NQT = HW // QT          # 32 q tiles
CH = 512                # matmul moving chunk (one PSUM bank of fp32)
NCHK = PSL // CH        # 2 chunks per core
SPLIT = 4               # q-tiles whose reduces run per-chunk (early start)

_F32 = mybir.dt.float32
_BF16 = mybir.dt.bfloat16
_AX = mybir.AxisListType
_AF = mybir.ActivationFunctionType
_OP = mybir.AluOpType


def _assign_engines():
    """Greedy tile->engine assignment by projected finish time.

    Costs are measured TimelineSim engine-busy ns per reduce instruction.
    Base loads account for head/extra work each engine owes during the
    main phase (DVE: absA+recips+combines, POOL: chunk-1 D mult spill).
    """
    full_cost = {"D": 1352.0, "A": 1259.0, "P": 1579.0}
    half_cost = {"D": 818.0, "A": 833.0, "P": 868.0}
    load = {"D": 2400.0, "A": 0.0, "P": 2400.0}
    halves = []  # (SPLIT tiles x 2 chunks)
    for _ in range(2 * SPLIT):
        e = min(load, key=lambda k: load[k] + half_cost[k])
        load[e] += half_cost[e]
        halves.append(e)
    fulls = []
    for _ in range(NQT - SPLIT):
        e = min(load, key=lambda k: load[k] + full_cost[k])
        load[e] += full_cost[e]
        fulls.append(e)
    return halves, fulls


def _emit(tc, a_d, bc_d, o_d):
    nc = tc.nc
    import contextlib

    halves, fulls = _assign_engines()

    with contextlib.ExitStack() as ctx:
        ctx.enter_context(
            nc.allow_low_precision(
                reason="bf16 matmul inputs; accumulation stays fp32"
            )
        )
        sb = ctx.enter_context(tc.tile_pool(name="sb", bufs=1))

        A = sb.tile([C, HW], _BF16)
        bc = sb.tile([C, 2 * PSL], _BF16)      # [b0|c0|b1|c1] 512-col chunks
        absBC = sb.tile([C, 2 * PSL], _BF16)
        absA = sb.tile([C, HW], _BF16)
        D = sb.tile([C, PSL], _BF16)
        t1 = sb.tile([C, CH], _BF16)
        t2 = sb.tile([C, CH], _BF16)
        ones_col = sb.tile([C, 1], _BF16)
        ones_row = sb.tile([1, C], _BF16)
        zeros_w = sb.tile([C, CH], _BF16)
        rr = sb.tile([C, 16], _BF16)           # 1/norm, partition-major
        rrT0 = sb.tile([8, C], _BF16)          # chunk-0 norms as rows
        rrT1 = sb.tile([8, C], _BF16)          # chunk-1 norms as rows
        onehots = sb.tile([8, 8 * QT], _BF16)  # K=8 row selectors for bcast
        rna = sb.tile([C, NQT], _F32)
        rs_d = sb.tile([C, NQT], _F32)
        rs_a = sb.tile([C, NQT], _F32)
        rs_p = sb.tile([C, NQT], _F32)
        rs_x = sb.tile([C, NQT], _F32)         # second halves of split tiles
        sum1 = sb.tile([C, NQT], _F32)
        sum2 = sb.tile([C, NQT], _F32)
        sum3 = sb.tile([C, NQT], _F32)
        res = sb.tile([C, NQT], _F32)
        trash_a = sb.tile([C, PSL], _BF16)
        trash_p = sb.tile([C, PSL], _BF16)

        # --- t=0: memsets + DMA issue + PE warmup -------------------------
        nc.vector.memset(zeros_w[:], 0.0)
        nc.vector.memset(ones_col[:], 1.0)
        nc.vector.memset(ones_row[:], 1.0)
        nc.vector.memset(rs_d[:], 0.0)
        nc.vector.memset(rs_a[:], 0.0)
        nc.vector.memset(rs_p[:], 0.0)
        nc.vector.memset(rs_x[:], 0.0)
        nc.vector.memset(onehots[:], 0.0)
        for u in range(8):
            nc.vector.memset(onehots[u : u + 1, u * QT : (u + 1) * QT], 1.0)

        # casting DMAs (gpsimd only): bc chunks first (gate the norm chain),
        # then a in 1024/1024/2048 pieces to feed the early q-tiles.
        nc.gpsimd.dma_start(bc[:, 0:1024], bc_d[:, 0:1024])
        nc.gpsimd.dma_start(bc[:, 1024:2048], bc_d[:, 1024:2048])
        nc.gpsimd.dma_start(A[:, 0:1024], a_d[:, 0:1024])
        nc.gpsimd.dma_start(A[:, 1024:2048], a_d[:, 1024:2048])
        nc.gpsimd.dma_start(A[:, 2048:4096], a_d[:, 2048:4096])

        with tc.tile_pool(name="warm_ps", bufs=1, space="PSUM") as warm_ps:
            warm = warm_ps.tile([C, CH], _F32)
            for _ in range(8):
                nc.tensor.matmul(
                    warm[:], lhsT=zeros_w[:, 0:QT], rhs=zeros_w[:],
                    start=True, stop=True,
                )

        # --- head: per 512-chunk norm chain -> D --------------------------
        # nbc col layout: j*8 + u (u<4: b block u, u>=4: c block u-4),
        # blocks are 128 columns of the p-slice chunk j.
        head_ctx = contextlib.ExitStack()
        nbc_ps = head_ctx.enter_context(tc.tile_pool(name="nbc_ps", bufs=1, space="PSUM"))
        rrt_ps = head_ctx.enter_context(tc.tile_pool(name="rrt_ps", bufs=2, space="PSUM"))
        bcst_ps = head_ctx.enter_context(tc.tile_pool(name="bcst_ps", bufs=2, space="PSUM"))
        nbc = nbc_ps.tile([C, 16], _F32)

        for j in range(NCHK):
            base = 1024 * j
            # |b|, |c| for this chunk: b on DVE (4x mode), c on ACT
            nc.vector.tensor_scalar(
                out=absBC[:, base : base + CH], in0=bc[:, base : base + CH],
                scalar1=0.0, scalar2=None, op0=_OP.abs_max,
            )
            nc.scalar.activation(
                absBC[:, base + CH : base + 1024],
                bc[:, base + CH : base + 1024], _AF.Abs, bias=0.0,
            )
            # column L1 norms, partition-major: 8 tiny N=1 matmuls
            for u in range(8):
                nc.tensor.matmul(
                    nbc[:, j * 8 + u : j * 8 + u + 1],
                    lhsT=absBC[:, base + u * QT : base + (u + 1) * QT],
                    rhs=ones_col[:],
                    start=True, stop=True,
                )
            # reciprocal in partition layout (cheap), then PE-transpose to rows
            nc.vector.reciprocal(rr[:, j * 8 : (j + 1) * 8], nbc[:, j * 8 : (j + 1) * 8])
            rrt = rrt_ps.tile([8, C], _BF16, tag="rrt")
            nc.tensor.matmul(
                rrt[:], lhsT=rr[:, j * 8 : (j + 1) * 8], rhs=zeros_w[:, 0:QT],
                start=True, stop=True, is_transpose=True,
            )
            rrT = rrT0 if j == 0 else rrT1
            nc.vector.tensor_copy(out=rrT[:], in_=rrt[:])
            # broadcast 1/nb, 1/nc rows across partitions: K=8 matmul with a
            # one-hot selector as the stationary tensor picks row u of rrT
            rb_bc = bcst_ps.tile([C, CH], _F32, tag="bcst")
            rc_bc = bcst_ps.tile([C, CH], _F32, tag="bcst")
            for u in range(4):
                nc.tensor.matmul(
                    rb_bc[:, u * QT : (u + 1) * QT],
                    lhsT=onehots[:, u * QT : (u + 1) * QT],
                    rhs=rrT[:],
                    start=True, stop=True,
                )
            for u in range(4):
                nc.tensor.matmul(
                    rc_bc[:, u * QT : (u + 1) * QT],
                    lhsT=onehots[:, (4 + u) * QT : (5 + u) * QT],
                    rhs=rrT[:],
                    start=True, stop=True,
                )
            # D_j = b*rb - c*rc   (t1 on DVE, t2 on Pool, sub on DVE)
            nc.vector.tensor_tensor(
                out=t1[:], in0=bc[:, base : base + CH], in1=rb_bc[:], op=_OP.mult
            )
            nc.gpsimd.tensor_tensor(
                out=t2[:], in0=bc[:, base + CH : base + 1024], in1=rc_bc[:], op=_OP.mult
            )
            nc.vector.tensor_tensor(
                out=D[:, j * CH : (j + 1) * CH], in0=t1[:], in1=t2[:], op=_OP.subtract
            )

        head_ctx.close()

        # --- main loop ----------------------------------------------------
        def emit_reduce(eng, m_ap, rs_tile, t):
            w = m_ap.shape[-1]
            if eng == "D":
                nc.vector.tensor_reduce(
                    out=rs_tile[:, t : t + 1], in_=m_ap, axis=_AX.X,
                    op=_OP.add, apply_absolute_value=True,
                )
            elif eng == "A":
                nc.scalar.activation(
                    trash_a[:, 0:w], m_ap, _AF.Abs, bias=0.0,
                    accum_out=rs_tile[:, t : t + 1],
                )
            else:
                nc.gpsimd.tensor_scalar(
                    out=trash_p[:, 0:w], in0=m_ap, scalar1=0.0, scalar2=None,
                    op0=_OP.abs_max, op1=_OP.add,
                    accum_out=rs_tile[:, t : t + 1],
                )

        def rs_of(eng):
            return {"D": rs_d, "A": rs_a, "P": rs_p}[eng]

        na_ps = ctx.enter_context(tc.tile_pool(name="na_ps", bufs=1, space="PSUM"))
        na = na_ps.tile([C, NQT], _F32)

        def emit_absA_na(k):
            # |a| chunk k (1024 cols; k=2 covers 2048) + its na matmuls
            lo = k * 1024
            hi = 4096 if k == 2 else lo + 1024
            nc.vector.tensor_scalar(
                out=absA[:, lo:hi], in0=A[:, lo:hi],
                scalar1=0.0, scalar2=None, op0=_OP.abs_max,
            )
            for t in range(lo // QT, hi // QT):
                nc.tensor.matmul(
                    na[:, t : t + 1],
                    lhsT=absA[:, t * QT : (t + 1) * QT],
                    rhs=ones_col[:],
                    start=True, stop=True,
                )

        with tc.tile_pool(name="m_split", bufs=4, space="PSUM") as m_split:
            # phase A: first SPLIT tiles on D chunk 0 only
            for t in range(SPLIT):
                M = m_split.tile([C, CH], _F32, tag="ms")
                nc.tensor.matmul(
                    M[:], lhsT=A[:, t * QT : (t + 1) * QT], rhs=D[:, 0:CH],
                    start=True, stop=True,
                )
                emit_reduce(halves[t], M[:], rs_of(halves[t]), t)
            # phase B: their D chunk 1 halves
            for t in range(SPLIT):
                M = m_split.tile([C, CH], _F32, tag="ms")
                nc.tensor.matmul(
                    M[:], lhsT=A[:, t * QT : (t + 1) * QT], rhs=D[:, CH:PSL],
                    start=True, stop=True,
                )
                e = halves[SPLIT + t]
                emit_reduce(e, M[:], rs_x if e == halves[t] else rs_of(e), t)

        # phase C: remaining tiles, full 1024-col reduces
        with tc.tile_pool(name="m_ps", bufs=3, space="PSUM") as m_ps:
            for i, t in enumerate(range(SPLIT, NQT)):
                if t == SPLIT + 1:
                    emit_absA_na(0)
                if t == SPLIT + 5:
                    emit_absA_na(1)
                if t == SPLIT + 13:
                    emit_absA_na(2)
                M = m_ps.tile([C, PSL], _F32, tag="m")
                for j in range(NCHK):
                    nc.tensor.matmul(
                        M[:, j * CH : (j + 1) * CH],
                        lhsT=A[:, t * QT : (t + 1) * QT],
                        rhs=D[:, j * CH : (j + 1) * CH],
                        start=True, stop=True,
                    )
                e = fulls[i]
                emit_reduce(e, M[:], rs_of(e), t)

            # --- tail: combine rowsums, scale by 1/na, store --------------
            nc.vector.reciprocal(rna[:], na[:])
            nc.gpsimd.tensor_tensor(out=sum2[:], in0=rs_p[:], in1=rs_x[:], op=_OP.add)
            nc.vector.tensor_tensor(out=sum1[:], in0=rs_d[:], in1=rs_a[:], op=_OP.add)
            nc.vector.tensor_tensor(out=sum3[:], in0=sum1[:], in1=sum2[:], op=_OP.add)
            nc.vector.tensor_tensor(out=res[:], in0=sum3[:], in1=rna[:], op=_OP.mult)
            nc.sync.dma_start(o_d, res[:])


def _build():
    nc = bacc.Bacc(
        "TRN2", target_bir_lowering=False, debug=False, num_devices=N_CORES
    )
    a_d = nc.dram_tensor("a_full", (C, HW), _F32, kind="ExternalInput").ap()
    bc_d = nc.dram_tensor("bc", (C, 2 * PSL), _F32, kind="ExternalInput").ap()
    o_d = nc.dram_tensor("out", (C, NQT), _F32, kind="ExternalOutput").ap()
    with tile.TileContext(nc) as tc:
        _emit(tc, a_d, bc_d, o_d)
    nc.finalize()
    return nc


_NC_CACHE = {}


def _get_nc():
    if "nc" not in _NC_CACHE:
        _NC_CACHE["nc"] = _build()
    return _NC_CACHE["nc"]


def _in_maps(a, b, c):
    a = np.ascontiguousarray(np.asarray(a, dtype=np.float32).reshape(B, C, HW))
    b = np.ascontiguousarray(np.asarray(b, dtype=np.float32).reshape(B, C, HW))
    c = np.ascontiguousarray(np.asarray(c, dtype=np.float32).reshape(B, C, HW))
    maps = []
    for core in range(N_CORES):
        bi, pi = divmod(core, 4)
        s0 = pi * PSL
        bc = np.concatenate(
            [
                b[bi, :, s0 : s0 + CH],
                c[bi, :, s0 : s0 + CH],
                b[bi, :, s0 + CH : s0 + PSL],
                c[bi, :, s0 + CH : s0 + PSL],
            ],
            axis=1,
        )
        maps.append(
            {
                "a_full": a[bi],
                "bc": np.ascontiguousarray(bc),
            }
        )
    return maps


def kernel(a, b, c):
    nc = _get_nc()
    res = bass_utils.run_bass_kernel_spmd(
        nc, _in_maps(a, b, c), core_ids=list(range(N_CORES))
    )
    total = np.float64(0.0)
    for core in range(N_CORES):
        total += np.sum(res.results[core]["out"], dtype=np.float64)
    return np.float32(total / (B * HW * HW))
